# revision 6
# baseline (speedup 1.0000x reference)
"""Trainium2 Bass kernel for multi-head attention (B=2, S=2048, D=1024, H=16).

Sharding: 8 cores = 2 (batch, data-parallel) x 4 (head-groups, tensor-parallel).
Each core (b, g) handles batch b and heads [4g, 4g+4), computing a partial
output contribution; the host sums the 4 head-group partials per batch and
adds the output bias.

Software-pipelined super-group loop: the 128 (qb, pair, kt) attention
groups are processed two at a time as [scores(i), scores(i+1), exp(i),
exp(i+1), attnV(i-2), attnV(i-1), extras], so the PE queue never stalls
waiting on the scalar engine's exp, and the scores<->attnV weight-switch
transition penalty (~110ns of exposed LDWEIGHTS + pipe refill at every
stationary-operand kind change after a row-packed pair) is paid once per
two groups. Host-side pre-arranged DRAM layouts give 1-descriptor-per-
partition DMAs (4-8KB bursts, cheap DIRECT2D descriptor generation).
Bulk DMAs ride only the two hardware-DGE queues (sync: k/v-side inputs +
outputs; scalar: q/v-side weights before the exp stream starts) -- the
gpsimd queue's software DGE is ~5x slower and carries only partition
broadcasts. ~6.4us of junk matmuls during the DMA lead-in hold the PE's
HAM clock gate at 2.4GHz so the first projection chains run warm; more
junk covers the tail's normalize bubble. The tail normalizes the last
pair in q-half chunks pipelined with the output projection.
Engines in steady state: ACT exp 1082ns/group (128x1024 from PSUM) vs PE
~1050ns/group -- near-lockstep, ~1.6% ACT stall. Phase profile at
~208us: lead-in ~24 (DMA-bandwidth-bound; the j0 input DMAs are halved
so projection chains start on the first half), qb0 ~60 (PE-bound: all
of kp/vp + half of qp project here), qb1-3 ~105.5 (ACT-bound), tail ~10
(q-half-chunked normalize + 4-deep output-unit PSUM pipelining across
the freed psA+psG banks, casts split across ACT+DVE).
"""

import os
import numpy as np
import ml_dtypes

import concourse.bass as bass
import concourse.bacc as bacc
import concourse.mybir as mybir
import concourse.tile as tile
from concourse.bass_utils import run_bass_kernel_spmd

F32 = mybir.dt.float32
BF16 = mybir.dt.bfloat16
AF = mybir.ActivationFunctionType

B, S, D = 2, 2048, 1024
H, DK = 16, 64
G = 4                  # head-groups (tensor parallel across cores)
DG = D // G            # 256 features per core
HPG = H // G           # 4 heads per core (2 pairs)
VEXT = HPG * (DK + 1)  # 260: per head [64 vp dims | 1 ones column]
H2 = DK + 1            # 65
P = 128
N_CORES = 8

_NC = None


def _build_program():
    nc = bacc.Bacc("TRN2", target_bir_lowering=False)
    # host-rearranged layouts: one contiguous run per partition per slice
    qTr = nc.dram_tensor("qTr", [P, 4, 8, 512], BF16, kind="ExternalInput")
    kTr = nc.dram_tensor("kTr", [P, 4, 8, 512], BF16, kind="ExternalInput")
    vTr = nc.dram_tensor("vTr", [P, 8, 8, 2 * P], BF16, kind="ExternalInput")
    wqr = nc.dram_tensor("wqr", [P, 2, 8, P], BF16, kind="ExternalInput")
    wkr = nc.dram_tensor("wkr", [P, 2, 8, P], BF16, kind="ExternalInput")
    wvr = nc.dram_tensor("wvr", [P, 8, VEXT], BF16, kind="ExternalInput")
    wvb = nc.dram_tensor("wvb", [1, VEXT], BF16, kind="ExternalInput")
    wor = nc.dram_tensor("wor", [P, 2, D], BF16, kind="ExternalInput")
    bqv = nc.dram_tensor("bqv", [P, 2], F32, kind="ExternalInput")
    bkv = nc.dram_tensor("bkv", [P, 2], F32, kind="ExternalInput")
    out = nc.dram_tensor("out", [S, D], BF16, kind="ExternalOutput")

    with tile.TileContext(nc) as tc:
        _body(nc, tc, qTr, kTr, vTr, wqr, wkr, wvr, wvb, wor, bqv, bkv, out)
    nc.compile()
    return nc


def _body(nc, tc, qTr, kTr, vTr, wqr, wkr, wvr, wvb, wor, bqv, bkv, out):
    with (
        tc.tile_pool(name="consts", bufs=1) as consts,
        tc.tile_pool(name="persist", bufs=1) as persist,
        tc.tile_pool(name="stage", bufs=7) as stage,
        tc.tile_pool(name="vstage", bufs=4) as vstage,
        tc.tile_pool(name="etp", bufs=6) as etp,
        tc.tile_pool(name="small", bufs=6) as small,
        tc.tile_pool(name="outp", bufs=8) as outp,
        tc.tile_pool(name="psG", bufs=2, space="PSUM") as psG,
        tc.tile_pool(name="psC", bufs=2, space="PSUM") as psC,
        tc.tile_pool(name="psA", bufs=2, space="PSUM") as psA,
    ):
        # ---- PE warm-up: the HAM clock gate needs ~3.4us of sustained matmul
        # activity to unthrottle 1.2->2.4GHz; burn junk matmuls during the
        # lead-in DMA wait so the first real chains run warm ----
        warmW = consts.tile([P, 512], BF16)
        nc.vector.memset(warmW[:], 0.0)
        # warm the ACT exp table early too (~2.7us load)
        warm = consts.tile([1, 8], F32)
        nc.vector.memset(warm[:], 0.0)
        nc.scalar.activation(warm[:], warm[:], AF.Exp)
        for _ in range(30):
            jp = psG.tile([P, 1024], F32, tag="g", name="jp")
            nc.tensor.matmul(jp[:, :256], lhsT=warmW[:, :P],
                             rhs=warmW[:, :256], start=True, stop=True)

        # ---- weights / inputs: first-chain dependencies lead each queue
        # (sync: k-side; gpsimd: q-side; vector: v-side). scalar queue
        # carries only exp. ----
        bk_sb = consts.tile([P, 2], F32)
        nc.sync.dma_start(bk_sb[:], bkv[:])
        wk_sb = consts.tile([P, 2, 8, P], BF16)
        nc.sync.dma_start(wk_sb[:, 0], wkr[:, 0])
        bq_sb = consts.tile([P, 2], F32)
        nc.scalar.dma_start(bq_sb[:], bqv[:])
        wq_sb = consts.tile([P, 2, 8, P], BF16)
        nc.scalar.dma_start(wq_sb[:, 0], wqr[:, 0])
        wvb_sb = consts.tile([1, VEXT], BF16)
        nc.scalar.dma_start(wvb_sb[:], wvb[:])
        wv_sb = consts.tile([P, 8, VEXT], BF16)
        nc.scalar.dma_start(wv_sb[:], wvr[:])
        wo_sb = consts.tile([P, 2, D], BF16)  # DMA deferred to qb0-pair1
        wvb_bc = consts.tile([P, VEXT], BF16)  # broadcast emitted post-bootstrap

        # ---- persistent activations ----
        qpT_sb = persist.tile([P, 2, S], BF16)   # [d%128, pair, s]
        kpT_sb = persist.tile([P, 2, S], BF16)
        vp_sb = persist.tile([P, 16, VEXT], BF16)  # [s%128, s-tile, 4*(64+1)]
        an_sb = persist.tile([P, 2, S], BF16)    # normalized attn output^T

        # ---- staged input loads ----
        XB = {}

        def load_xb(name, src, j, eng=None, split=False):
            xb = stage.tile([P, 8, 512], BF16, tag="xb", name="xb")
            e = eng or nc.sync
            if split:
                # halved transfers: the projection chain's first 4 matmuls
                # start as soon as the first half lands (subtile deps)
                e.dma_start(xb[:, 0:4], src[:, j, 0:4])
                e.dma_start(xb[:, 4:8], src[:, j, 4:8])
            else:
                e.dma_start(xb[:], src[:, j])
            XB[(name, j)] = xb

        VTB = {}

        def load_vtb(st2, eng=None):
            vtb = vstage.tile([P, 8, 2 * P], BF16, tag="vtb", name="vtb")
            (eng or nc.sync).dma_start(vtb[:], vTr[:, st2])
            VTB[st2] = vtb

        # ---- projection chains (qp/kp: N=512, LDW hides under streaming) ----
        def proj_chain(name, j, w_sb, b_sb, dst, dt):
            xb = XB[(name, j)]
            ps = psA.tile([P, 512], F32, tag="a", name="pproj")
            for kt in range(8):
                nc.tensor.matmul(
                    ps[:],
                    lhsT=w_sb[:, dt, kt, :],
                    rhs=xb[:, kt, :],
                    start=(kt == 0),
                    stop=(kt == 7),
                )
            nc.vector.tensor_scalar_add(
                dst[:, dt, j * 512 : (j + 1) * 512], ps[:], b_sb[:, dt : dt + 1]
            )

        # ---- vp: data-stationary N=260 matmuls, fed one at a time so their
        # LDWEIGHTS hide under neighboring N=512 matmuls ----
        vp_queue = []        # tiles waiting to start
        vp_cur = None        # (st, psv, next_kt)

        def vp_enqueue(st):
            vp_queue.append(st)

        def vp_feed(n):
            nonlocal vp_cur
            for _ in range(n):
                if vp_cur is None:
                    if not vp_queue:
                        return
                    st = vp_queue.pop(0)
                    psv = psA.tile([P, 512], F32, tag="a", name="pv")
                    vp_cur = [st, psv, 0]
                st, psv, kt = vp_cur
                vtb = VTB[st // 2]
                off = (st % 2) * P
                nc.tensor.matmul(
                    psv[:, :VEXT],
                    lhsT=vtb[:, kt, off : off + P],
                    rhs=wv_sb[:, kt, :],
                    start=(kt == 0),
                    stop=(kt == 7),
                )
                if kt == 7:
                    nc.vector.tensor_tensor(
                        vp_sb[:, st, :], psv[:, :VEXT], wvb_bc[:],
                        mybir.AluOpType.add,
                    )
                    vp_cur = None
                else:
                    vp_cur[2] = kt + 1

        def vp_drain_until(st_needed):
            while vp_cur is not None and vp_cur[0] <= st_needed:
                vp_feed(1)
            while vp_queue and vp_queue[0] <= st_needed:
                vp_feed(1)
                while vp_cur is not None:
                    vp_feed(1)

        # ---- attention groups ----
        GROUPS = [(qb, pair, kt) for qb in range(4) for pair in range(2)
                  for kt in range(16)]
        ET = {}
        GPS = {}
        CC = {}

        def scores_mm(i):
            qb, pair, kt = GROUPS[i]
            qs = slice(qb * 512, (qb + 1) * 512)
            gps = psG.tile([P, 1024], F32, tag="g", name="gps")
            for hh in range(2):
                hp = slice(hh * DK, (hh + 1) * DK)
                nc.tensor.matmul(
                    gps[:, hh * 512 : (hh + 1) * 512],
                    lhsT=kpT_sb[hp, pair, kt * P : (kt + 1) * P],
                    rhs=qpT_sb[hp, pair, qs],
                    start=True,
                    stop=True,
                )
                vp_feed(1)
            GPS[i] = gps

        def exp_act(i):
            et = etp.tile([P, 1024], BF16, tag="e", name="et")
            nc.scalar.activation(et[:], GPS.pop(i)[:], AF.Exp,
                                 scale=1.0 / np.sqrt(DK))
            ET[i] = et

        def attn_mms(i):
            qb, pair, kt = GROUPS[i]
            vp_drain_until(kt)
            if kt == 0:
                CC[(qb, pair)] = [
                    psC.tile([H2, 512], F32, tag="c", name="cc") for _ in range(2)
                ]
            cc = CC[(qb, pair)]
            et = ET.pop(i)
            for hh in range(2):
                h = 2 * pair + hh
                nc.tensor.matmul(
                    cc[hh][:],
                    lhsT=vp_sb[:, kt, h * H2 : (h + 1) * H2],
                    rhs=et[:, hh * 512 : (hh + 1) * 512],
                    start=(kt == 0),
                    stop=(kt == 15),
                )
                vp_feed(1)
            if kt == 15 and (qb, pair) != (3, 1):
                # release the accumulator banks with fast DVE copies; the
                # normalize chains run from SBUF during the next two groups
                CSB[(qb, pair)] = []
                for hh in range(2):
                    csb = small.tile([H2, 512], F32, tag="csb")
                    nc.vector.tensor_copy(csb[:], cc[hh][:])
                    CSB[(qb, pair)].append(csb)
                del CC[(qb, pair)]

        CSB = {}

        def norm_half(qb, pair, hh, src=None, coff=None):
            qs = slice(qb * 512, (qb + 1) * 512)
            csb = src if src is not None else CSB[(qb, pair)][hh]
            rsum = small.tile([1, 512], F32, tag="rsum")
            nc.vector.tensor_copy(rsum[:], csb[DK : DK + 1, :])
            rinv = small.tile([1, 512], F32, tag="rinv")
            nc.vector.reciprocal_approx_fast(rinv[:], rsum[:])
            rbc = small.tile([DK, 512], F32, tag="rbc")
            nc.gpsimd.partition_broadcast(rbc[:], rinv[:])
            nc.vector.tensor_tensor(
                an_sb[hh * DK : (hh + 1) * DK, pair, qs],
                csb[:DK, :],
                rbc[:],
                mybir.AluOpType.mult,
            )

        def d_unit(qb, u, tail=False):
            qt, o = u // 2, u % 2
            q0 = qb * 512 + qt * P
            if tail and u % 2:
                # psG's banks are free after the last exp: 4-deep dps
                # pipelining so unit k+2 never waits on unit k's cast
                dps = psG.tile([P, 1024], F32, tag="g", name="dpsg")[:, :512]
            else:
                dps = psA.tile([P, 512], F32, tag="a", name="dps")
            for p2 in range(2):
                nc.tensor.matmul(
                    dps[:],
                    lhsT=an_sb[:, p2, q0 : q0 + P],
                    rhs=wo_sb[:, p2, o * 512 : (o + 1) * 512],
                    start=(p2 == 0),
                    stop=(p2 == 1),
                )
                vp_feed(1)
            osb = outp.tile([P, 512], BF16, tag="o")
            if tail and u % 2 == 0:
                # ACT is idle after the last exp: split the tail casts
                # across both engines to halve the serial cast chain
                nc.scalar.copy(osb[:], dps[:])
            else:
                nc.vector.tensor_copy(osb[:], dps[:])
            nc.sync.dma_start(out[q0 : q0 + P, o * 512 : (o + 1) * 512], osb[:])

        # ---- extras schedule: i -> list of thunks ----
        EX = {i: [] for i in range(128)}

        def at(i, fn, *a):
            EX[i].append((fn, a))

        # qb0-pair0 (groups 0..15): input DMAs + kp/qp chains + vp feed
        at(0, load_xb, 'k', kTr, 2)
        at(0, load_vtb, 2)
        at(1, load_vtb, 3)
        at(2, load_xb, 'k', kTr, 3)
        at(2, proj_chain, 'k', 1, wk_sb, bk_sb, kpT_sb, 0)
        at(3, load_vtb, 4)
        at(5, load_vtb, 5)
        at(5, proj_chain, 'k', 2, wk_sb, bk_sb, kpT_sb, 0)
        at(7, load_vtb, 6)
        at(8, load_xb, 'q', qTr, 1)
        at(8, proj_chain, 'k', 3, wk_sb, bk_sb, kpT_sb, 0)
        at(9, load_vtb, 7)
        at(10, proj_chain, 'k', 0, wk_sb, bk_sb, kpT_sb, 1)
        at(12, proj_chain, 'q', 0, wq_sb, bq_sb, qpT_sb, 1)
        def vp_boot():
            vp_enqueue(0)
            vp_enqueue(1)
            vp_drain_until(1)

        at(1, vp_boot)
        for st in range(2, 16):
            at(st - 2, vp_enqueue, st)

        # pair0 normalize runs in the first groups of the same-qb pair1 block
        for qb in range(4):
            at(32 * qb + 17, norm_half, qb, 0, 0)
            at(32 * qb + 18, norm_half, qb, 0, 1)

        # qb0-pair1 (groups 16..31)
        at(16, lambda: nc.sync.dma_start(wo_sb[:], wor[:]))
        at(17, proj_chain, 'k', 1, wk_sb, bk_sb, kpT_sb, 1)
        at(20, proj_chain, 'k', 2, wk_sb, bk_sb, kpT_sb, 1)
        at(23, proj_chain, 'k', 3, wk_sb, bk_sb, kpT_sb, 1)
        at(26, proj_chain, 'q', 1, wq_sb, bq_sb, qpT_sb, 0)

        # steady qbs
        for qb in range(1, 4):
            b0 = 32 * qb          # pair0 block start
            b1 = 32 * qb + 16     # pair1 block start
            at(b0 + 1, norm_half, qb - 1, 1, 0)
            at(b0 + 2, norm_half, qb - 1, 1, 1)
            # qp dt1 for this qb's pair1 runs early in its pair0 block
            at(b0 + 3, proj_chain, 'q', qb, wq_sb, bq_sb, qpT_sb, 1)
            for k, u in zip((5, 8, 11, 14), range(4)):
                at(b0 + k, d_unit, qb - 1, u)
            for k, u in zip((2, 5, 8, 11), range(4, 8)):
                at(b1 + k, d_unit, qb - 1, u)
            if qb < 3:
                at(b0, load_xb, 'q', qTr, qb + 1)
                at(b1 + 1, proj_chain, 'q', qb + 1, wq_sb, bq_sb, qpT_sb, 0)

        # ---- bootstrap ----
        load_xb('k', kTr, 0, nc.sync, split=True)
        load_xb('q', qTr, 0, nc.sync, split=True)
        nc.scalar.dma_start(wq_sb[:, 1], wqr[:, 1])
        load_vtb(0)
        load_xb('k', kTr, 1, nc.sync)
        load_vtb(1, nc.scalar)
        nc.sync.dma_start(wk_sb[:, 1], wkr[:, 1])
        nc.gpsimd.partition_broadcast(wvb_bc[:], wvb_sb[:])
        proj_chain('k', 0, wk_sb, bk_sb, kpT_sb, 0)
        proj_chain('q', 0, wq_sb, bq_sb, qpT_sb, 0)

        # ---- main pipelined loop: super-groups of 2 kt halve the
        # scores<->attnV weight-switch transitions on the PE ----
        for sg in range(64):
            i0, i1 = 2 * sg, 2 * sg + 1
            scores_mm(i0)
            scores_mm(i1)
            exp_act(i0)
            exp_act(i1)
            if sg > 0:
                attn_mms(i0 - 2)
                attn_mms(i1 - 2)
            for i in (i0, i1):
                for fn, a in EX[i]:
                    fn(*a)
        attn_mms(126)
        attn_mms(127)

        # ---- tail: chunked normalize (q-halves) pipelined with the output
        # projection; junk matmuls keep the HAM clock warm across the
        # normalize bubble ----
        cc3 = CC[(3, 1)]
        njunk = [0]

        def tail_junk(n):
            for _ in range(n):
                jp2 = psG.tile([P, 1024], F32, tag="g", name="jp2")
                nc.tensor.matmul(jp2[:, :512], lhsT=warmW[:, :P],
                                 rhs=warmW[:], start=True, stop=True)

        tail_junk(6)
        for qc in range(2):
            cs = slice(qc * 256, (qc + 1) * 256)
            qsc = slice(3 * 512 + qc * 256, 3 * 512 + (qc + 1) * 256)
            for hh in range(2):
                rsum = small.tile([1, 256], F32, tag="rsum")
                nc.vector.tensor_copy(rsum[:], cc3[hh][DK : DK + 1, cs])
                rinv = small.tile([1, 256], F32, tag="rinv")
                nc.vector.reciprocal_approx_fast(rinv[:], rsum[:])
                rbc = small.tile([DK, 256], F32, tag="rbc")
                nc.gpsimd.partition_broadcast(rbc[:], rinv[:])
                nc.vector.tensor_tensor(
                    an_sb[hh * DK : (hh + 1) * DK, 1, qsc],
                    cc3[hh][:DK, cs],
                    rbc[:],
                    mybir.AluOpType.mult,
                )
            if qc == 0:
                tail_junk(5)
        for u in range(8):
            d_unit(3, u, tail=True)


def _get_program():
    global _NC
    if _NC is None:
        _NC = _build_program()
    return _NC


def _make_in_maps(v, k, q, Wv, bv, Wk, bk, Wq, bq, Wo, bo):
    f32 = np.float32
    bf16 = ml_dtypes.bfloat16

    def xpose(x):
        # [S, D] -> [P, 4j, 8t, 512s] with x.T[t*128+p, j*512+s]
        xT = np.ascontiguousarray(x.T)                  # [D, S]
        return np.ascontiguousarray(
            xT.reshape(8, P, 4, 512).transpose(1, 2, 0, 3)
        ).astype(bf16)

    def vpose(x):
        xT = np.ascontiguousarray(x.T)                  # [D, S]
        return np.ascontiguousarray(
            xT.reshape(8, P, 8, 2 * P).transpose(1, 2, 0, 3)
        ).astype(bf16)

    qTr = [xpose(q[b]) for b in range(B)]
    kTr = [xpose(k[b]) for b in range(B)]
    vTr = [vpose(v[b]) for b in range(B)]

    per_group = []
    for g in range(G):
        gs = slice(g * DG, (g + 1) * DG)
        # [D, DG] -> [P, 8t, DG]
        wqr = np.ascontiguousarray(
            Wq[gs, :].T.reshape(8, P, 2, P).transpose(1, 2, 0, 3)
        ).astype(bf16)
        wkr = np.ascontiguousarray(
            Wk[gs, :].T.reshape(8, P, 2, P).transpose(1, 2, 0, 3)
        ).astype(bf16)
        wvm = np.zeros((D, VEXT), dtype=f32)
        wvb = np.zeros((1, VEXT), dtype=f32)
        for h in range(HPG):
            cs = slice(h * H2, h * H2 + DK)
            rows = slice(g * DG + h * DK, g * DG + (h + 1) * DK)
            wvm[:, cs] = Wv[rows, :].T
            wvb[0, cs] = bv[rows]
            wvb[0, h * H2 + DK] = 1.0
        wvr = np.ascontiguousarray(
            wvm.reshape(8, P, VEXT).transpose(1, 0, 2)
        ).astype(bf16)
        wvb = wvb.astype(bf16)
        wor = np.ascontiguousarray(
            Wo[:, gs].T.reshape(2, P, D).transpose(1, 0, 2)
        ).astype(bf16)
        per_group.append(
            dict(
                wqr=wqr,
                wkr=wkr,
                wvr=wvr,
                wvb=wvb,
                wor=wor,
                bqv=np.ascontiguousarray(bq[gs].reshape(2, P).T, dtype=f32),
                bkv=np.ascontiguousarray(bk[gs].reshape(2, P).T, dtype=f32),
            )
        )

    in_maps = []
    for c in range(N_CORES):
        b, g = c // G, c % G
        m = dict(qTr=qTr[b], kTr=kTr[b], vTr=vTr[b], **per_group[g])
        in_maps.append(m)
    return in_maps


def _gather(results, bo):
    out = np.zeros((B, S, D), dtype=np.float32)
    for c in range(N_CORES):
        b = c // G
        out[b] += np.asarray(results[c]["out"], dtype=np.float32)
    out += bo.astype(np.float32)
    return out


def run(v, k, q, Wv, bv, Wk, bk, Wq, bq, Wo, bo, trace=False):
    nc = _get_program()
    in_maps = _make_in_maps(v, k, q, Wv, bv, Wk, bk, Wq, bq, Wo, bo)
    res = run_bass_kernel_spmd(
        nc, in_maps, core_ids=list(range(N_CORES)), trace=trace
    )
    return _gather(res.results, np.asarray(bo)), res


def kernel(v, k, q, Wv, bv, Wk, bk, Wq, bq, Wo, bo):
    args = [np.asarray(x, dtype=np.float32)
            for x in (v, k, q, Wv, bv, Wk, bk, Wq, bq, Wo, bo)]
    out, _ = run(*args, trace=bool(int(os.environ.get("MHA_TRACE", "0"))))
    return out
